# revision 7
# baseline (speedup 1.0000x reference)
"""Trainium2 Bass kernel for nn_Encoder (NRI-style GNN message-passing encoder).

Reference math:
  h  = MLP1(x)                       [B,N,H]   N=64 nodes, H=128
  e  = MLP2(node2edge(h))            [B,E,H]   E=4032 edges (fully connected)
  n  = MLP3(edge2node(e))            [B,N,H]
  e2 = MLP4([node2edge(n), e])       [B,E,H]
  out= e2 @ wout + bout              [B,E,16]

Distribution: data-parallel over batch, 8 items per core x 8 cores.

Kernel structure (all-bf16, fp32 PSUM accumulation):
- Edge reorder (receiver-major): reordered edge p = (s-1)*64 + j is the
  edge (sender=(j+s)%64, receiver=j), s=1..63.
- node2edge for MLP2 is LOW-RANK: per-node projections A = W2s.h + b2a,
  B = W2r.h (64-col matmuls), expanded per edge as
  h2pre[:, (s-1)*64+j] = A2[:, s+j] + B[:, j] with a single DVE
  tensor_tensor add (A2 = [A A]; shifted window via stride-1 outer AP,
  B via stride-0 broadcast AP). ReLU in-place via tensor_scalar_max.
- edge2node: halving add-tree; levels 1-2 on DVE (2x bf16 mode),
  levels 3-6 on the otherwise-idle GpSimd.
- MLP4 pre-activation in PSUM: w2bk @ h2T (per-edge matmul) + node term
  via ONE matmul per chunk: [AnT;BnT] stacked [128,128] stationary
  against a constant one-hot incidence matrix onehot2 [128, E]
  (onehot2[i(e),e]=1, onehot2[64+j(e),e]=1).  AnT/BnT are produced
  TRANSPOSED directly by using n1T as the matmul stationary
  (out = n1T.T @ w34s) with tile_position row-stacking, so no
  duplication/shift tricks are needed for the node term.
- All per-edge MLP4 biases are constant per hidden dim, so they fold
  into the ReLU-evac bias (b4sr); the PSUM evacs do relu+bias in one
  instruction (3 on ACT, 1 on GpSimd per item).
- Output layer: 8 chunks packed DENSELY into one [128,512] fp32 PSUM
  bank per item using two stationaries [w4o|0] and [0|w4o] [128,32]:
  chunk c lands at partition rows 32*(c%4)+16*(c//4). One GpSimd copy
  evacuates [128,512]->bf16, one DMA per item (half the baseline
  output bytes).
- Every interior linear layer is folded host-side (exact algebra):
  w12s/w12r = w1b@w2a halves, w23 = scale*(w2b@w3a), w34s/w34r =
  w3b@w4a halves, w2bk = w2b@w4a_k, w4o = w4b@wout; biases folded to
  match; b4o added on host.
- 3-stage software pipeline across items; stage3 big matmuls are
  interleaved with stage2 smalls (stage3 steps emitted first so PE is
  never head-of-line blocked on the tree-dependent stage2 matmuls).

The harness calls kernel(**inputs) with full unsharded inputs.
"""
import sys

sys.path.insert(0, "/opt/trn_rl_repo")

import numpy as np
import ml_dtypes

import concourse.bass as bass
from concourse import bacc
import concourse.mybir as mybir
import concourse.tile as tile
from concourse.bass_utils import run_bass_kernel_spmd

F32 = mybir.dt.float32
BF16 = mybir.dt.bfloat16
BFNP = ml_dtypes.bfloat16

N_NODES = 64
N_EDGES = 4032
BATCH = 64
N_IN = 64
H = 128
N_OUT = 16
N_CORES = 8
B_LOC = BATCH // N_CORES

# chunk c covers s-blocks d0..d0+7 (edge cols (d0-1)*64 .. +512); chunk 7
# overlaps chunk 6 by one block so every chunk is exactly 512 columns.
CHUNKS = [1 + 8 * c for c in range(7)] + [56]

# wpack layout (bf16): [xT(512) | w1a | w12s | w12r | w23 | w34sr(256) |
#                       w2bk | w4oz2(64) | onehot2(4032)]
XCOLS = B_LOC * N_NODES            # 512
OFF_W1A = XCOLS                    # 512:640
OFF_W12S = OFF_W1A + H             # 640:768
OFF_W12R = OFF_W12S + H            # 768:896
OFF_W23 = OFF_W12R + H             # 896:1024
OFF_W34 = OFF_W23 + H              # 1024:1280 (w34s | w34r)
OFF_W2BK = OFF_W34 + 2 * H         # 1280:1408
OFF_W4OZ = OFF_W2BK + H            # 1408:1472 ([w4o|0] | [0|w4o])
OFF_OH = OFF_W4OZ + 64             # 1472:5504
WTOT = OFF_OH + N_EDGES            # 5504
BNAMES = ["b1a", "b2a", "b2r", "b23", "b4sr"]

_AF = mybir.ActivationFunctionType
_ALU = mybir.AluOpType


def _edge_perm():
    """perm[p] = original edge index of reordered edge p = (s-1)*64 + j,
    which is edge (sender=(j+s)%64, receiver=j)."""
    s, j = np.meshgrid(np.arange(1, 64), np.arange(64), indexing="ij")
    i = (j + s) % 64
    return (i * 63 + (j - (j > i))).reshape(-1)


def _ap(t, off, dims):
    return bass.AP(tensor=t.tensor, offset=t.offset + off, ap=[t.ap[0]] + dims)


def build_kernel():
    nc = bacc.Bacc("TRN2", target_bir_lowering=False, debug=False)

    wpack_d = nc.dram_tensor("wpack", [H, WTOT], BF16, kind="ExternalInput").ap()
    bias_d = nc.dram_tensor("bias", [H, 8], F32, kind="ExternalInput").ap()
    # per item: [128, 512] bf16; chunk c at partition rows
    # 32*(c%4)+16*(c//4) .. +16, transposed (rows=outs, cols=edges).
    y_d = nc.dram_tensor("y", [B_LOC, H, 512], BF16, kind="ExternalOutput").ap()

    with tile.TileContext(nc) as tc:
        with (
            tc.tile_pool(name="wp", bufs=1) as wp,
            tc.tile_pool(name="hp", bufs=1) as hp,       # h1T/A2all/Ball
            tc.tile_pool(name="h2p", bufs=4) as h2p,     # h2T per item
            tc.tile_pool(name="trp", bufs=3) as trp,     # tree scratch
            tc.tile_pool(name="smp", bufs=3) as smp,     # n1T
            tc.tile_pool(name="nbp", bufs=4) as nbp,     # AnBnT per item
            tc.tile_pool(name="h4p", bufs=6) as h4p,     # MLP4 activations
            tc.tile_pool(name="osp", bufs=3) as osp,     # packed output
            tc.tile_pool(name="psml", bufs=2, space="PSUM") as psml,   # 1 bank
            tc.tile_pool(name="pbig", bufs=2, space="PSUM") as pbig,   # 2 banks
            tc.tile_pool(name="ppo", bufs=2, space="PSUM") as ppo,     # 1 bank
        ):
            wall = wp.tile([H, WTOT], BF16)
            bt = wp.tile([H, 8], F32)
            # biases first (tiny), then the MLP1 slice so it can start early
            nc.sync.dma_start(bt, bias_d)
            nc.sync.dma_start(wall[:, 0:768], wpack_d[:, 0:768])
            nc.sync.dma_start(wall[:, 768:OFF_OH], wpack_d[:, 768:OFF_OH])
            nc.sync.dma_start(wall[:, OFF_OH:WTOT], wpack_d[:, OFF_OH:WTOT])
            xT = wall[0:N_IN, 0:XCOLS]
            w1a = wall[0:N_IN, OFF_W1A:OFF_W1A + H]
            w12s = wall[:, OFF_W12S:OFF_W12S + H]
            w12r = wall[:, OFF_W12R:OFF_W12R + H]
            w23 = wall[:, OFF_W23:OFF_W23 + H]
            w34s = wall[:, OFF_W34:OFF_W34 + H]
            w34r = wall[:, OFF_W34 + H:OFF_W34 + 2 * H]
            w2bk = wall[:, OFF_W2BK:OFF_W2BK + H]
            w4lo = wall[:, OFF_W4OZ:OFF_W4OZ + 32]
            w4hi = wall[:, OFF_W4OZ + 32:OFF_W4OZ + 64]
            onehot = wall[:, OFF_OH:OFF_OH + N_EDGES]
            b = {n: bt[:, i:i + 1] for i, n in enumerate(BNAMES)}

            # ---- MLP1 layer 1 over all 512 tokens ----
            p1 = psml.tile([H, XCOLS], F32, tag="psml")
            nc.tensor.matmul(p1, w1a, xT, start=True, stop=True)
            h1T = hp.tile([H, XCOLS], BF16, tag="h1T")
            nc.scalar.activation(h1T, p1, _AF.Relu, bias=b["b1a"])

            # ---- A/B node projections (MLP1-L2 folded in), all items ----
            pA = psml.tile([H, XCOLS], F32, tag="psml")
            nc.tensor.matmul(pA, w12s, h1T, start=True, stop=True)
            pB = psml.tile([H, XCOLS], F32, tag="psml")
            nc.tensor.matmul(pB, w12r, h1T, start=True, stop=True)
            # A2all: per item the 64 A-columns duplicated -> [A_b A_b]
            A2all = hp.tile([H, 2 * XCOLS], BF16, tag="A2all")
            nc.scalar.activation(
                _ap(A2all, 0, [[128, B_LOC], [64, 2], [1, 64]]),
                _ap(pA, 0, [[64, B_LOC], [0, 2], [1, 64]]),
                _AF.Identity, bias=b["b2a"],
            )
            Ball = hp.tile([H, XCOLS], BF16, tag="Ball")
            nc.scalar.activation(Ball, pB, _AF.Identity, bias=b["b2r"])

            def stage1(bb):
                """edge expansion + relu + tree levels 1-2 (DVE)."""
                h2T = h2p.tile([H, N_EDGES], BF16, tag="h2T")
                nc.vector.tensor_tensor(
                    _ap(h2T, 0, [[64, 63], [1, 64]]),
                    _ap(A2all, 128 * bb + 1, [[1, 63], [1, 64]]),
                    _ap(Ball, 64 * bb, [[0, 63], [1, 64]]),
                    _ALU.add,
                )
                nc.vector.tensor_scalar_max(h2T[:, :], h2T[:, :], 0.0)
                T = trp.tile([H, 2048], BF16, tag="T")
                nc.vector.tensor_add(
                    T[:, 0:1984], h2T[:, 0:1984], h2T[:, 2048:4032])
                nc.vector.tensor_copy(T[:, 1984:2048], h2T[:, 1984:2048])
                nc.vector.tensor_add(T[:, 0:1024], T[:, 0:1024], T[:, 1024:2048])
                return h2T, T

            def tree_tail(bb, T):
                """Tree levels 3-6.  GpSimd normally (emitted at END of the
                cycle so it doesn't head-of-line-block the h4/outS evacs);
                DVE for item 0 to shorten the pipeline-fill chain."""
                eng = nc.vector if bb == 0 else nc.gpsimd
                ww = 512
                while ww >= N_NODES:
                    eng.tensor_add(T[:, 0:ww], T[:, 0:ww], T[:, ww:2 * ww])
                    ww //= 2

            def stage2(bb, T, out):
                """edge2node tail + MLP3 + transposed An/Bn projections.
                Generator: yields between serial steps so the emitter can
                interleave stage3 big matmuls into the PE/ACT queues."""
                pn1 = psml.tile([H, N_NODES], F32, tag="psml")
                nc.tensor.matmul(pn1, w23, T[:, 0:N_NODES],
                                 start=True, stop=True)
                n1T = smp.tile([H, N_NODES], BF16, tag="n1T")
                nc.scalar.activation(n1T, pn1, _AF.Relu, bias=b["b23"])
                yield
                # AnBnT[n, h'] rows 0:64 = (w34s.T n1).T, rows 64:128 =
                # (w34r.T n1).T -- n1T as STATIONARY gives the transpose.
                pABn = psml.tile([H, H], F32, tag="psml")
                nc.tensor.matmul(pABn[0:64, :], n1T, w34s,
                                 start=True, stop=True, tile_position=(0, 0))
                nc.tensor.matmul(pABn[64:128, :], n1T, w34r,
                                 start=True, stop=True, tile_position=(0, 64))
                yield
                AnBnT = nbp.tile([H, H], BF16, tag="AnBnT")
                nc.vector.tensor_copy(AnBnT, pABn)
                out[bb] = AnBnT

            def stage3(bb, h2T, AnBnT):
                """MLP4 + folded output layer for item bb.
                Generator: yields between matmul groups for interleaving."""
                outS = osp.tile([H, 512], BF16, tag="outS")
                po = ppo.tile([H, 512], F32, tag="ppo")
                for half in range(2):
                    pms = [pbig.tile([H, 1024], F32, tag="pbig", name="pm4")
                           for _ in range(2)]
                    cs = [4 * half + i for i in range(4)]
                    for i, c in enumerate(cs):
                        e0 = (CHUNKS[c] - 1) * 64
                        nc.tensor.matmul(
                            pms[i // 2][:, 512 * (i % 2):512 * (i % 2 + 1)],
                            w2bk, h2T[:, e0:e0 + 512],
                            start=True, stop=False, skip_group_check=True,
                        )
                    yield
                    for i, c in enumerate(cs):
                        e0 = (CHUNKS[c] - 1) * 64
                        nc.tensor.matmul(
                            pms[i // 2][:, 512 * (i % 2):512 * (i % 2 + 1)],
                            AnBnT, onehot[:, e0:e0 + 512],
                            start=False, stop=True, skip_group_check=True,
                        )
                    yield
                    h4s = []
                    for i in range(2):
                        h4 = h4p.tile([H, 1024], BF16, tag="h4")
                        nc.scalar.activation(h4, pms[i], _AF.Relu,
                                             bias=b["b4sr"])
                        h4s.append(h4)
                    yield
                    for i, c in enumerate(cs):
                        r0 = 32 * (c % 4)
                        nc.tensor.matmul(
                            po[r0:r0 + 32, :],
                            w4lo if half == 0 else w4hi,
                            h4s[i // 2][:, 512 * (i % 2):512 * (i % 2 + 1)],
                            start=(half == 0), stop=(half == 1),
                            tile_position=(0, r0), skip_group_check=True,
                        )
                    yield
                # GpSimd cannot read PSUM; alternate the po evac between
                # DVE and ACT to balance the two engines.
                if bb % 2 == 0:
                    nc.vector.tensor_copy(outS, po)
                else:
                    nc.scalar.activation(outS, po, _AF.Identity, bias=0.0)
                nc.sync.dma_start(y_d[bb], outS)

            # ---- software pipeline: stage1(b) | stage2(b-1) | stage3(b-2);
            # stage3 steps are emitted before stage2 steps so PE is not
            # head-of-line blocked on the tree-dependent pn1 matmul.
            s1 = {}
            s2 = {}
            for cyc in range(B_LOC + 2):
                if cyc < B_LOC:
                    s1[cyc] = stage1(cyc)
                g2 = g3 = None
                if 1 <= cyc <= B_LOC:
                    g2 = stage2(cyc - 1, s1[cyc - 1][1], s2)
                if cyc >= 2:
                    bb = cyc - 2
                    g3 = stage3(bb, s1.pop(bb)[0], s2.pop(bb))
                while True:
                    done = True
                    if g3 is not None:
                        try:
                            next(g3)
                            done = False
                        except StopIteration:
                            g3 = None
                    if g2 is not None:
                        try:
                            next(g2)
                            done = False
                        except StopIteration:
                            g2 = None
                    if done:
                        break
                if cyc < B_LOC:
                    tree_tail(cyc, s1[cyc][1])

    nc.compile()
    return nc


_CACHE = {}


def _get_nc():
    if "nc" not in _CACHE:
        _CACHE["nc"] = build_kernel()
        _CACHE["perm"] = _edge_perm()
    return _CACHE["nc"], _CACHE["perm"]


def _onehot2():
    p = np.arange(N_EDGES)
    s = p // 64 + 1
    j = p % 64
    i = (j + s) % 64
    oh = np.zeros((H, N_EDGES), np.float32)
    oh[i, p] = 1.0
    oh[64 + j, p] = 1.0
    return oh


def make_in_maps(inputs):
    w2b = np.asarray(inputs["w2b"], np.float32)
    w4a = np.asarray(inputs["w4a"], np.float32)
    b2b = np.asarray(inputs["b2b"], np.float32)
    w4a_k = w4a[2 * H:]

    def pad128(a):
        out = np.zeros((H, a.shape[1]), np.float32)
        out[:a.shape[0]] = a
        return out

    w4o = np.asarray(inputs["w4b"], np.float32) @ inputs["wout"]
    w1b = np.asarray(inputs["w1b"], np.float32)
    b1b = np.asarray(inputs["b1b"], np.float32)
    w2a = np.asarray(inputs["w2a"], np.float32)
    w3a = np.asarray(inputs["w3a"], np.float32)
    w3b = np.asarray(inputs["w3b"], np.float32)
    b3b = np.asarray(inputs["b3b"], np.float32)
    scale2n = 1.0 / (63.0 + 1e-6)
    w4oz2 = np.zeros((H, 64), np.float32)
    w4oz2[:, 0:N_OUT] = w4o           # [w4o | 0]
    w4oz2[:, 48:64] = w4o             # [0 | w4o]
    wblocks = [
        pad128(np.asarray(inputs["w1a"], np.float32)),
        w1b @ w2a[:H], w1b @ w2a[H:],
        scale2n * (w2b @ w3a),
        w3b @ w4a[:H], w3b @ w4a[H:2 * H],
        w2b @ w4a_k,
        w4oz2,
        _onehot2(),
    ]
    bcols = [
        inputs["b1a"],
        b1b @ w2a[:H] + inputs["b2a"], b1b @ w2a[H:],
        (63.0 * scale2n) * (b2b @ w3a) + inputs["b3a"],
        (b3b @ w4a[:H] + inputs["b4a"] + b2b @ w4a_k
         + b3b @ w4a[H:2 * H]),
        np.zeros(H, np.float32), np.zeros(H, np.float32),
        np.zeros(H, np.float32),
    ]
    wfix = np.concatenate(
        [np.ascontiguousarray(v, np.float32) for v in wblocks], axis=1)
    bias = np.stack([np.asarray(v, np.float32) for v in bcols], axis=1)
    bias = np.ascontiguousarray(bias)
    x = np.asarray(inputs["x"], np.float32)
    in_maps = []
    for c in range(N_CORES):
        xs = x[c * B_LOC:(c + 1) * B_LOC]
        xTp = pad128(xs.reshape(B_LOC * N_NODES, N_IN).T)
        wpack = np.concatenate([xTp, wfix], axis=1).astype(BFNP)
        in_maps.append({"wpack": np.ascontiguousarray(wpack), "bias": bias})
    return in_maps


def gather_out(results, perm, inputs):
    b4o = (np.asarray(inputs["b4b"], np.float32) @ inputs["wout"]
           + inputs["bout"]).astype(np.float32)  # [16]
    inv = np.empty_like(perm)
    inv[perm] = np.arange(N_EDGES)
    out = np.empty((BATCH, N_EDGES, N_OUT), np.float32)
    full = np.empty((B_LOC, N_EDGES, N_OUT), np.float32)
    for cr in range(N_CORES):
        y = np.asarray(results[cr]["y"]).astype(np.float32)  # [B_LOC,128,512]
        for c in range(8):
            r0 = 32 * (c % 4) + 16 * (c // 4)
            e0 = (CHUNKS[c] - 1) * 64
            seg = y[:, r0:r0 + N_OUT, :]
            full[:, e0:e0 + 512, :] = seg.transpose(0, 2, 1)
        out[cr * B_LOC:(cr + 1) * B_LOC] = full[:, inv, :] + b4o
    return out


def kernel(**inputs):
    nc, perm = _get_nc()
    in_maps = make_in_maps(inputs)
    res = run_bass_kernel_spmd(nc, in_maps, core_ids=list(range(N_CORES)))
    return gather_out(res.results, perm, inputs)


# revision 10
# speedup vs baseline: 1.2521x; 1.2521x over previous
"""Trainium2 Bass kernel for nn_Encoder (NRI-style GNN message-passing encoder).

Reference math:
  h  = MLP1(x)                       [B,N,H]   N=64 nodes, H=128
  e  = MLP2(node2edge(h))            [B,E,H]   E=4032 edges (fully connected)
  n  = MLP3(edge2node(e))            [B,N,H]
  e2 = MLP4([node2edge(n), e])       [B,E,H]
  out= e2 @ wout + bout              [B,E,16]

Distribution: data-parallel over batch, 8 items per core x 8 cores.

Kernel structure (all-bf16, fp32 PSUM accumulation):
- Edge reorder (receiver-major): reordered edge p = (s-1)*64 + j is the
  edge (sender=(j+s)%64, receiver=j), s=1..63.
- node2edge for MLP2 is LOW-RANK: per-node projections A = W2s.h + b2a,
  B = W2r.h (64-col matmuls), expanded per edge as
  h2pre[:, (s-1)*64+j] = A2[:, s+j] + B[:, j] with a single DVE
  tensor_tensor add (A2 = [A A]; shifted window via stride-1 outer AP,
  B via stride-0 broadcast AP). ReLU in-place via tensor_scalar_max.
- edge2node: halving add-tree; levels 1-2 on DVE (2x bf16 mode),
  levels 3-6 on the otherwise-idle GpSimd.
- MLP4 pre-activation in PSUM: w2bk @ h2T (per-edge matmul) + node term
  via ONE matmul per chunk: [AnT;BnT] stacked [128,128] stationary
  against a constant one-hot incidence matrix onehot2 [128, E]
  (onehot2[i(e),e]=1, onehot2[64+j(e),e]=1).  AnT/BnT are produced
  TRANSPOSED directly by using n1T as the matmul stationary
  (out = n1T.T @ w34s) with tile_position row-stacking, so no
  duplication/shift tricks are needed for the node term.
- All per-edge MLP4 biases are constant per hidden dim, so they fold
  into the ReLU-evac bias (b4sr); the PSUM evacs do relu+bias in one
  instruction (3 on ACT, 1 on GpSimd per item).
- Output layer: 8 chunks packed DENSELY into one [128,512] fp32 PSUM
  bank per item using two stationaries [w4o|0] and [0|w4o] [128,32]:
  chunk c lands at partition rows 32*(c%4)+16*(c//4). One GpSimd copy
  evacuates [128,512]->bf16, one DMA per item (half the baseline
  output bytes).
- Every interior linear layer is folded host-side (exact algebra):
  w12s/w12r = w1b@w2a halves, w23 = scale*(w2b@w3a), w34s/w34r =
  w3b@w4a halves, w2bk = w2b@w4a_k, w4o = w4b@wout; biases folded to
  match; b4o added on host.
- 3-stage software pipeline across items; stage3 big matmuls are
  interleaved with stage2 smalls (stage3 steps emitted first so PE is
  never head-of-line blocked on the tree-dependent stage2 matmuls).

The harness calls kernel(**inputs) with full unsharded inputs.
"""
import sys

sys.path.insert(0, "/opt/trn_rl_repo")

import numpy as np
import ml_dtypes

import concourse.bass as bass
from concourse import bacc
import concourse.mybir as mybir
import concourse.tile as tile
from concourse.bass_utils import run_bass_kernel_spmd

F32 = mybir.dt.float32
BF16 = mybir.dt.bfloat16
BFNP = ml_dtypes.bfloat16

N_NODES = 64
N_EDGES = 4032
BATCH = 64
N_IN = 64
H = 128
N_OUT = 16
N_CORES = 8
B_LOC = BATCH // N_CORES

# chunk c covers s-blocks d0..d0+7 (edge cols (d0-1)*64 .. +512); chunk 7
# overlaps chunk 6 by one block so every chunk is exactly 512 columns.
CHUNKS = [1 + 8 * c for c in range(7)] + [56]

# wpack layout (bf16): [xT(512) | w1a | w12s | w12r | w23 | w34sr(256) |
#                       w2bk | w4oz2(64) | onehot2(4032)]
XCOLS = B_LOC * N_NODES            # 512
OFF_W1A = XCOLS                    # 512:640
OFF_W12S = OFF_W1A + H             # 640:768
OFF_W12R = OFF_W12S + H            # 768:896
OFF_W23 = OFF_W12R + H             # 896:1024
OFF_W34 = OFF_W23 + H              # 1024:1280 (w34s | w34r)
OFF_W2BK = OFF_W34 + 2 * H         # 1280:1408
OFF_W4OZ = OFF_W2BK + H            # 1408:1472 ([w4o|0] | [0|w4o])
OFF_OH = OFF_W4OZ + 64             # 1472:5504
WTOT = OFF_OH + N_EDGES            # 5504
BNAMES = ["b1a", "b2a", "b2r", "b23", "b4sr"]

_AF = mybir.ActivationFunctionType
_ALU = mybir.AluOpType


def _edge_perm():
    """perm[p] = original edge index of reordered edge p = (s-1)*64 + j,
    which is edge (sender=(j+s)%64, receiver=j)."""
    s, j = np.meshgrid(np.arange(1, 64), np.arange(64), indexing="ij")
    i = (j + s) % 64
    return (i * 63 + (j - (j > i))).reshape(-1)


def _ap(t, off, dims):
    return bass.AP(tensor=t.tensor, offset=t.offset + off, ap=[t.ap[0]] + dims)


def build_kernel():
    nc = bacc.Bacc("TRN2", target_bir_lowering=False, debug=False)

    wpack_d = nc.dram_tensor("wpack", [H, WTOT], BF16, kind="ExternalInput").ap()
    bias_d = nc.dram_tensor("bias", [H, 8], F32, kind="ExternalInput").ap()
    # per item: [128, 512] bf16; chunk c at partition rows
    # 32*(c%4)+16*(c//4) .. +16, transposed (rows=outs, cols=edges).
    y_d = nc.dram_tensor("y", [B_LOC, H, 512], BF16, kind="ExternalOutput").ap()

    with tile.TileContext(nc) as tc:
        with (
            tc.tile_pool(name="wp", bufs=1) as wp,
            tc.tile_pool(name="hp", bufs=1) as hp,       # h1T/A2all/Ball
            tc.tile_pool(name="h2p", bufs=4) as h2p,     # h2T per item
            tc.tile_pool(name="trp", bufs=3) as trp,     # tree scratch
            tc.tile_pool(name="smp", bufs=3) as smp,     # n1T
            tc.tile_pool(name="nbp", bufs=4) as nbp,     # AnBnT per item
            tc.tile_pool(name="h4p", bufs=6) as h4p,     # MLP4 activations
            tc.tile_pool(name="osp", bufs=3) as osp,     # packed output
            tc.tile_pool(name="psml", bufs=2, space="PSUM") as psml,   # 1 bank
            tc.tile_pool(name="pbig", bufs=2, space="PSUM") as pbig,   # 2 banks
            tc.tile_pool(name="ppo", bufs=2, space="PSUM") as ppo,     # 1 bank
        ):
            wall = wp.tile([H, WTOT], BF16)
            bt = wp.tile([H, 8], F32)
            # biases first (tiny), then the MLP1 slice so it can start early
            nc.sync.dma_start(bt, bias_d)
            nc.sync.dma_start(wall[:, 0:768], wpack_d[:, 0:768])
            nc.sync.dma_start(wall[:, 768:OFF_OH], wpack_d[:, 768:OFF_OH])
            nc.sync.dma_start(wall[:, OFF_OH:WTOT], wpack_d[:, OFF_OH:WTOT])
            xT = wall[0:N_IN, 0:XCOLS]
            w1a = wall[0:N_IN, OFF_W1A:OFF_W1A + H]
            w12s = wall[:, OFF_W12S:OFF_W12S + H]
            w12r = wall[:, OFF_W12R:OFF_W12R + H]
            w23 = wall[:, OFF_W23:OFF_W23 + H]
            w34s = wall[:, OFF_W34:OFF_W34 + H]
            w34r = wall[:, OFF_W34 + H:OFF_W34 + 2 * H]
            w2bk = wall[:, OFF_W2BK:OFF_W2BK + H]
            w4lo = wall[:, OFF_W4OZ:OFF_W4OZ + 32]
            w4hi = wall[:, OFF_W4OZ + 32:OFF_W4OZ + 64]
            onehot = wall[:, OFF_OH:OFF_OH + N_EDGES]
            b = {n: bt[:, i:i + 1] for i, n in enumerate(BNAMES)}

            # ---- MLP1 layer 1 over all 512 tokens ----
            p1 = psml.tile([H, XCOLS], F32, tag="psml")
            nc.tensor.matmul(p1, w1a, xT, start=True, stop=True)
            h1T = hp.tile([H, XCOLS], BF16, tag="h1T")
            nc.scalar.activation(h1T, p1, _AF.Relu, bias=b["b1a"])

            # ---- A/B node projections (MLP1-L2 folded in), all items ----
            pA = psml.tile([H, XCOLS], F32, tag="psml")
            nc.tensor.matmul(pA, w12s, h1T, start=True, stop=True)
            pB = psml.tile([H, XCOLS], F32, tag="psml")
            nc.tensor.matmul(pB, w12r, h1T, start=True, stop=True)
            # A2all: per item the 64 A-columns duplicated -> [A_b A_b]
            A2all = hp.tile([H, 2 * XCOLS], BF16, tag="A2all")
            nc.scalar.activation(
                _ap(A2all, 0, [[128, B_LOC], [64, 2], [1, 64]]),
                _ap(pA, 0, [[64, B_LOC], [0, 2], [1, 64]]),
                _AF.Identity, bias=b["b2a"],
            )
            Ball = hp.tile([H, XCOLS], BF16, tag="Ball")
            nc.scalar.activation(Ball, pB, _AF.Identity, bias=b["b2r"])

            def stage1(bb):
                """edge expansion + relu + tree levels 1-2 (DVE)."""
                h2T = h2p.tile([H, N_EDGES], BF16, tag="h2T")
                nc.vector.tensor_tensor(
                    _ap(h2T, 0, [[64, 63], [1, 64]]),
                    _ap(A2all, 128 * bb + 1, [[1, 63], [1, 64]]),
                    _ap(Ball, 64 * bb, [[0, 63], [1, 64]]),
                    _ALU.add,
                )
                nc.vector.tensor_scalar_max(h2T[:, :], h2T[:, :], 0.0)
                T = trp.tile([H, 2048], BF16, tag="T")
                nc.vector.tensor_add(
                    T[:, 0:1984], h2T[:, 0:1984], h2T[:, 2048:4032])
                nc.vector.tensor_copy(T[:, 1984:2048], h2T[:, 1984:2048])
                ww = 1024
                while ww >= N_NODES:
                    nc.vector.tensor_add(T[:, 0:ww], T[:, 0:ww], T[:, ww:2 * ww])
                    ww //= 2
                return h2T, T

            def stage2(bb, T, out):
                """edge2node tail + MLP3 + transposed An/Bn projections.
                Generator: yields between serial steps so the emitter can
                interleave stage3 big matmuls into the PE/ACT queues."""
                pn1 = psml.tile([H, N_NODES], F32, tag="psml")
                nc.tensor.matmul(pn1, w23, T[:, 0:N_NODES],
                                 start=True, stop=True)
                n1T = smp.tile([H, N_NODES], BF16, tag="n1T")
                nc.scalar.activation(n1T, pn1, _AF.Relu, bias=b["b23"])
                yield
                # AnBnT[n, h'] rows 0:64 = (w34s.T n1).T, rows 64:128 =
                # (w34r.T n1).T -- n1T as STATIONARY gives the transpose.
                pABn = psml.tile([H, H], F32, tag="psml")
                nc.tensor.matmul(pABn[0:64, :], n1T, w34s,
                                 start=True, stop=True, tile_position=(0, 0))
                nc.tensor.matmul(pABn[64:128, :], n1T, w34r,
                                 start=True, stop=True, tile_position=(0, 64))
                yield
                AnBnT = nbp.tile([H, H], BF16, tag="AnBnT")
                nc.scalar.activation(AnBnT, pABn, _AF.Identity, bias=0.0)
                out[bb] = AnBnT

            def stage3(bb, h2T, AnBnT):
                """MLP4 + folded output layer for item bb.
                Generator: yields between matmul groups for interleaving."""
                outS = osp.tile([H, 512], BF16, tag="outS")
                po = ppo.tile([H, 512], F32, tag="ppo")
                for half in range(2):
                    pms = [pbig.tile([H, 1024], F32, tag="pbig", name="pm4")
                           for _ in range(2)]
                    cs = [4 * half + i for i in range(4)]
                    for i, c in enumerate(cs):
                        e0 = (CHUNKS[c] - 1) * 64
                        nc.tensor.matmul(
                            pms[i // 2][:, 512 * (i % 2):512 * (i % 2 + 1)],
                            w2bk, h2T[:, e0:e0 + 512],
                            start=True, stop=False, skip_group_check=True,
                        )
                    yield
                    for i, c in enumerate(cs):
                        e0 = (CHUNKS[c] - 1) * 64
                        nc.tensor.matmul(
                            pms[i // 2][:, 512 * (i % 2):512 * (i % 2 + 1)],
                            AnBnT, onehot[:, e0:e0 + 512],
                            start=False, stop=True, skip_group_check=True,
                        )
                    yield
                    h4s = []
                    for i in range(2):
                        h4 = h4p.tile([H, 1024], BF16, tag="h4")
                        nc.scalar.activation(h4, pms[i], _AF.Relu,
                                             bias=b["b4sr"])
                        h4s.append(h4)
                    yield
                    for i, c in enumerate(cs):
                        r0 = 32 * (c % 4)
                        nc.tensor.matmul(
                            po[r0:r0 + 32, :],
                            w4lo if half == 0 else w4hi,
                            h4s[i // 2][:, 512 * (i % 2):512 * (i % 2 + 1)],
                            start=(half == 0), stop=(half == 1),
                            tile_position=(0, r0), skip_group_check=True,
                        )
                    yield
                # GpSimd cannot read PSUM; alternate the po evac between
                # DVE and ACT to balance the two engines.
                if bb % 2 == 0:
                    nc.vector.tensor_copy(outS, po)
                else:
                    nc.scalar.activation(outS, po, _AF.Identity, bias=0.0)
                nc.sync.dma_start(y_d[bb], outS)

            # ---- software pipeline: stage1(b) | stage2(b-1) | stage3(b-2);
            # stage3 steps are emitted before stage2 steps so PE is not
            # head-of-line blocked on the tree-dependent pn1 matmul.
            s1 = {}
            s2 = {}
            for cyc in range(B_LOC + 2):
                if cyc < B_LOC:
                    s1[cyc] = stage1(cyc)
                g2 = g3 = None
                if 1 <= cyc <= B_LOC:
                    g2 = stage2(cyc - 1, s1[cyc - 1][1], s2)
                if cyc >= 2:
                    bb = cyc - 2
                    g3 = stage3(bb, s1.pop(bb)[0], s2.pop(bb))
                while True:
                    done = True
                    if g3 is not None:
                        try:
                            next(g3)
                            done = False
                        except StopIteration:
                            g3 = None
                    if g2 is not None:
                        try:
                            next(g2)
                            done = False
                        except StopIteration:
                            g2 = None
                    if done:
                        break


    nc.compile()
    return nc


_CACHE = {}


def _get_nc():
    if "nc" not in _CACHE:
        _CACHE["nc"] = build_kernel()
        _CACHE["perm"] = _edge_perm()
    return _CACHE["nc"], _CACHE["perm"]


def _onehot2():
    p = np.arange(N_EDGES)
    s = p // 64 + 1
    j = p % 64
    i = (j + s) % 64
    oh = np.zeros((H, N_EDGES), np.float32)
    oh[i, p] = 1.0
    oh[64 + j, p] = 1.0
    return oh


def make_in_maps(inputs):
    w2b = np.asarray(inputs["w2b"], np.float32)
    w4a = np.asarray(inputs["w4a"], np.float32)
    b2b = np.asarray(inputs["b2b"], np.float32)
    w4a_k = w4a[2 * H:]

    def pad128(a):
        out = np.zeros((H, a.shape[1]), np.float32)
        out[:a.shape[0]] = a
        return out

    w4o = np.asarray(inputs["w4b"], np.float32) @ inputs["wout"]
    w1b = np.asarray(inputs["w1b"], np.float32)
    b1b = np.asarray(inputs["b1b"], np.float32)
    w2a = np.asarray(inputs["w2a"], np.float32)
    w3a = np.asarray(inputs["w3a"], np.float32)
    w3b = np.asarray(inputs["w3b"], np.float32)
    b3b = np.asarray(inputs["b3b"], np.float32)
    scale2n = 1.0 / (63.0 + 1e-6)
    w4oz2 = np.zeros((H, 64), np.float32)
    w4oz2[:, 0:N_OUT] = w4o           # [w4o | 0]
    w4oz2[:, 48:64] = w4o             # [0 | w4o]
    wblocks = [
        pad128(np.asarray(inputs["w1a"], np.float32)),
        w1b @ w2a[:H], w1b @ w2a[H:],
        scale2n * (w2b @ w3a),
        w3b @ w4a[:H], w3b @ w4a[H:2 * H],
        w2b @ w4a_k,
        w4oz2,
        _onehot2(),
    ]
    bcols = [
        inputs["b1a"],
        b1b @ w2a[:H] + inputs["b2a"], b1b @ w2a[H:],
        (63.0 * scale2n) * (b2b @ w3a) + inputs["b3a"],
        (b3b @ w4a[:H] + inputs["b4a"] + b2b @ w4a_k
         + b3b @ w4a[H:2 * H]),
        np.zeros(H, np.float32), np.zeros(H, np.float32),
        np.zeros(H, np.float32),
    ]
    wfix = np.concatenate(
        [np.ascontiguousarray(v, np.float32) for v in wblocks], axis=1)
    bias = np.stack([np.asarray(v, np.float32) for v in bcols], axis=1)
    bias = np.ascontiguousarray(bias)
    x = np.asarray(inputs["x"], np.float32)
    in_maps = []
    for c in range(N_CORES):
        xs = x[c * B_LOC:(c + 1) * B_LOC]
        xTp = pad128(xs.reshape(B_LOC * N_NODES, N_IN).T)
        wpack = np.concatenate([xTp, wfix], axis=1).astype(BFNP)
        in_maps.append({"wpack": np.ascontiguousarray(wpack), "bias": bias})
    return in_maps


def gather_out(results, perm, inputs):
    b4o = (np.asarray(inputs["b4b"], np.float32) @ inputs["wout"]
           + inputs["bout"]).astype(np.float32)  # [16]
    inv = np.empty_like(perm)
    inv[perm] = np.arange(N_EDGES)
    out = np.empty((BATCH, N_EDGES, N_OUT), np.float32)
    full = np.empty((B_LOC, N_EDGES, N_OUT), np.float32)
    for cr in range(N_CORES):
        y = np.asarray(results[cr]["y"]).astype(np.float32)  # [B_LOC,128,512]
        for c in range(8):
            r0 = 32 * (c % 4) + 16 * (c // 4)
            e0 = (CHUNKS[c] - 1) * 64
            seg = y[:, r0:r0 + N_OUT, :]
            full[:, e0:e0 + 512, :] = seg.transpose(0, 2, 1)
        out[cr * B_LOC:(cr + 1) * B_LOC] = full[:, inv, :] + b4o
    return out


def kernel(**inputs):
    nc, perm = _get_nc()
    in_maps = make_in_maps(inputs)
    res = run_bass_kernel_spmd(nc, in_maps, core_ids=list(range(N_CORES)))
    return gather_out(res.results, perm, inputs)
